# revision 34
# baseline (speedup 1.0000x reference)
"""DGCNN (2x EdgeConv kNN=5 + MLP head) Trainium2 kernel, data-parallel over 8 NeuronCores.

Contract: kernel(**inputs) takes the FULL inputs of nn_DEC_41180146434796
(pos [32,2048,3] + MLP weights) and returns the FULL [32,2] output.
Each core processes 4 graphs end-to-end (kNN, gathers, max-aggregations local).

v2 design notes:
- The DVE top-k (max8 + find_index8 over [128,2048] gram tiles) is the hard
  floor (~75us per conv-graph). Everything else is moved off the DVE
  (conv adds/maxes -> Pool, relu/casts -> ACT, BN folds -> weights) and the
  4 graphs are software-pipelined: each graph's conv/MLP phase is emitted
  fine-grained-interleaved into the NEXT top-k phase's gram tiles so PSUM
  ring slot priority matches the intended overlap.
- Gram operands drop the per-row-constant |x_i|^2 (rank-preserving):
  lhs = [2x; 1], rhs = [x; -|x_j|^2].
- find_index8 results are redistributed to the gather's wrapped layout with
  a PE transpose + one-hot replication matmuls (no serial DMA storm).
"""
import numpy as np

import concourse.bass as bass
import concourse.mybir as mybir
from concourse import bacc, tile
from concourse import bass_utils
from concourse.masks import make_identity

F32 = mybir.dt.float32
F32R = mybir.dt.float32r
F16 = mybir.dt.float16
U32 = mybir.dt.uint32
I16 = mybir.dt.int16
AF = mybir.ActivationFunctionType
ALU = mybir.AluOpType
AX = mybir.AxisListType

N = 2048          # nodes per graph
NG = 4            # graphs per core
K = 5             # kNN neighbors (incl self)
NT = 16           # node tiles of 128
NCORES = 8

_CACHE = {}


def _sigma_read(ap):
    """View a [C, 2048] natural-ordered tensor so its free stream is sigma-ordered.

    sigma col s = 16*q + b  <->  node i = 128*b + q.
    """
    return ap.rearrange("c (b q) -> c q b", b=16, q=128)


def build_nc():
    nc = bacc.Bacc(None, target_bir_lowering=False)

    # ---------------- I/O ----------------
    posT_d = nc.dram_tensor("posT", [NG, 36, N], F32, kind="ExternalInput")
    w1aA_d = nc.dram_tensor("w1aA", [3, 64], F32, kind="ExternalInput")
    w1aB_d = nc.dram_tensor("w1aB", [3, 64], F32, kind="ExternalInput")
    w1b_d = nc.dram_tensor("w1b", [64, 64], F16, kind="ExternalInput")
    w1c_d = nc.dram_tensor("w1c", [64, 64], F16, kind="ExternalInput")
    w2A_d = nc.dram_tensor("w2A", [64, 128], F16, kind="ExternalInput")
    w2B_d = nc.dram_tensor("w2B", [64, 128], F16, kind="ExternalInput")
    wl1_d = nc.dram_tensor("wl1", [64, 1024], F16, kind="ExternalInput")
    wl2_d = nc.dram_tensor("wl2", [128, 1024], F16, kind="ExternalInput")
    wm1_d = nc.dram_tensor("wm1", [128, 8, 512], F16, kind="ExternalInput")
    wm2_d = nc.dram_tensor("wm2", [128, 4, 256], F16, kind="ExternalInput")
    wout_d = nc.dram_tensor("wout", [128, 2, 2], F16, kind="ExternalInput")
    rep4_d = nc.dram_tensor("rep4", [64, 4, 64], F32, kind="ExternalInput")
    rep8_d = nc.dram_tensor("rep8", [64, 4, 128], F32, kind="ExternalInput")
    w1aT_d = nc.dram_tensor("w1aT", [3, 64], F32, kind="ExternalInput")
    b1a_d = nc.dram_tensor("b1a", [64, 1], F32, kind="ExternalInput")
    b1b_d = nc.dram_tensor("b1b", [64, 1], F32, kind="ExternalInput")
    b1c_d = nc.dram_tensor("b1c", [64, 1], F32, kind="ExternalInput")
    s1c_d = nc.dram_tensor("s1c", [64, 1], F32, kind="ExternalInput")
    s1c2_d = nc.dram_tensor("s1c2", [64, 1], F32, kind="ExternalInput")
    b2_d = nc.dram_tensor("b2", [128, 1], F32, kind="ExternalInput")
    bl_d = nc.dram_tensor("bl", [128, 8], F32, kind="ExternalInput")
    bm1_d = nc.dram_tensor("bm1", [128, 4], F32, kind="ExternalInput")
    bm2_d = nc.dram_tensor("bm2", [128, 2], F32, kind="ExternalInput")
    bout_d = nc.dram_tensor("bout", [2, 1], F32, kind="ExternalInput")

    out_d = nc.dram_tensor("out", [2, NG], F32, kind="ExternalOutput")

    with tile.TileContext(nc) as tc:
        with tc.tile_pool(name="wpool", bufs=1) as wp, \
             tc.tile_pool(name="persist", bufs=1) as pp, \
             tc.tile_pool(name="work", bufs=1) as wk, \
             tc.tile_pool(name="ps", bufs=2, space="PSUM") as psp:

            W = {}

            def wload_all():
                def wload(dram, shape, dtype, name):
                    t = wp.tile(shape, dtype, name=name)
                    if dtype == F32R:
                        nc.sync.dma_start(t[:].bitcast(F32), dram[:])
                    else:
                        nc.sync.dma_start(t[:], dram[:])
                    return t

                def wload_r(dram, shape, name):
                    tmp = wk.tile(shape, F32, tag="idxf", bufs=2, name=name + "_stg")
                    nc.sync.dma_start(tmp[:], dram[:])
                    t = wp.tile(shape, F32R, name=name)
                    nc.scalar.activation(t[:], tmp[:], AF.Copy)
                    return t

                W["w1aA"] = wload_r(w1aA_d, [3, 64], "w1aA_s")
                W["w1aB"] = wload(w1aB_d, [3, 64], F32, "w1aB_s")
                W["w1b"] = wload(w1b_d, [64, 64], F16, "w1b_s")
                W["w1c"] = wload(w1c_d, [64, 64], F16, "w1c_s")
                W["w2A"] = wload(w2A_d, [64, 128], F16, "w2A_s")
                W["w2B"] = wload(w2B_d, [64, 128], F16, "w2B_s")
                W["wl1"] = wload(wl1_d, [64, 1024], F16, "wl1_s")
                W["wl2"] = wload(wl2_d, [128, 1024], F16, "wl2_s")
                W["wm1"] = wload(wm1_d, [128, 8, 512], F16, "wm1_s")
                W["wm2"] = wload(wm2_d, [128, 4, 256], F16, "wm2_s")
                W["wout"] = wload(wout_d, [128, 2, 2], F16, "wout_s")
                W["rep4"] = wload(rep4_d, [64, 4, 64], F32, "rep4_s")
                W["rep8"] = wload(rep8_d, [64, 4, 128], F32, "rep8_s")
                W["w1aT"] = wload_r(w1aT_d, [3, 64], "w1aT_s")
                W["b1a"] = wload(b1a_d, [64, 1], F32, "b1a_s")
                W["b1b"] = wload(b1b_d, [64, 1], F32, "b1b_s")
                W["b1c"] = wload(b1c_d, [64, 1], F32, "b1c_s")
                W["s1c"] = wload(s1c_d, [64, 1], F32, "s1c_s")
                W["s1c2"] = wload(s1c2_d, [64, 1], F32, "s1c2_s")
                W["b2"] = wload(b2_d, [128, 1], F32, "b2_s")
                W["bl"] = wload(bl_d, [128, 8], F32, "bl_s")
                W["bm1"] = wload(bm1_d, [128, 4], F32, "bm1_s")
                W["bm2"] = wload(bm2_d, [128, 2], F32, "bm2_s")
                W["bout"] = wload(bout_d, [2, 1], F32, "bout_s")

            ident = wp.tile([128, 128], F32, name="ident")
            make_identity(nc, ident[:])
            ones64 = wp.tile([64, 1], F16, name="ones64")
            nc.vector.memset(ones64[:], 1.0)

            # pooled & relu'd features for the head: [128, mt(8), graph(4)]
            poolr = pp.tile([128, 8, NG], F16, name="poolr")

            S = [dict() for _ in range(NG)]

            # ---------------- stage chunk builders ----------------
            def chunks_A(g):
                def th():
                    posTmp = wk.tile([36, N], F32, tag="posT", bufs=2,
                                     name=f"posTmp{g}")
                    nc.sync.dma_start(posTmp[:], posT_d[g])
                    # host rows: [x; -|x|^2] at 0-3 (also the 16-ch gather
                    # source), [x; 0.5] at 32-35 (engine reads must start at
                    # 0/32/64); ACT copies round to f32r for the gram matmuls
                    rhsA = wk.tile([4, N], F32R, tag="rhsA", bufs=2,
                                   name=f"rhsA{g}")
                    nc.scalar.activation(rhsA[:], posTmp[0:4, :], AF.Copy)
                    lhsA = wk.tile([4, N], F32R, tag="lhsA", bufs=2,
                                   name=f"lhsA{g}")
                    nc.scalar.activation(lhsA[:], posTmp[32:36, :], AF.Copy,
                                         scale=2.0)
                    S[g].update(posTmp=posTmp, rhsA=rhsA, lhsA=lhsA)
                return [th]

            def _wrap_thunk(g, idx_tag, ngroups):
                def th():
                    idxall = S[g][idx_tag]
                    idxf = wk.tile([128, 4, 16], F32, tag="idxf", bufs=2,
                                   name=f"idxf_{idx_tag}{g}")
                    nc.vector.tensor_copy(idxf[:],
                                          idxall[:, :, 1:5].transpose([0, 2, 1]))
                    tp = psp.tile([64, 128], F32, tag="ps", name=f"tp{idx_tag}{g}")
                    nc.tensor.transpose(tp[:], idxf[:].rearrange("p a b -> p (a b)"),
                                        ident[:])
                    idxtf = wk.tile([64, 128], F32, tag="idxtf", bufs=2,
                                    name=f"idxtf_{idx_tag}{g}")
                    nc.scalar.activation(idxtf[:], tp[:], AF.Copy)
                    rep = W["rep4"] if ngroups == 4 else W["rep8"]
                    wrapP = psp.tile([16 * ngroups, 512], F32, tag="ps",
                                     name=f"wrapP{idx_tag}{g}")
                    for k in range(4):
                        nc.tensor.matmul(wrapP[:, 128 * k:128 * (k + 1)],
                                         rep[:, k, :], idxtf[:])
                    wrap = wk.tile([16 * ngroups, 512], I16, tag=f"wrap{ngroups}",
                                   bufs=2, name=f"wrap{ngroups}_{g}")
                    nc.scalar.activation(wrap[:], wrapP[:], AF.Copy)
                    S[g][f"wrap{1 if ngroups == 4 else 2}"] = wrap
                return th

            def chunks_TK(g, conv):
                lhs_key = "lhsA" if conv == 1 else "lhsA2"
                rhs_key = "rhsA" if conv == 1 else "rhsA2"
                idx_tag = f"idx{conv}"
                thunks = []
                # idxall tile created in first chunk
                def mk(t):
                    def th():
                        if t == 0:
                            S[g][idx_tag] = wk.tile([128, NT, 8], U32, tag=idx_tag,
                                                    bufs=2, name=f"{idx_tag}_{g}")
                        idxall = S[g][idx_tag]
                        lhs, rhs = S[g][lhs_key], S[g][rhs_key]
                        ps = psp.tile([128, N], F32, tag="ps",
                                      name=f"gps{conv}_{g}_{t}")
                        for c in range(4):
                            nc.tensor.matmul(ps[:, 512 * c:512 * (c + 1)],
                                             lhs[:, 128 * t:128 * (t + 1)],
                                             rhs[:, 512 * c:512 * (c + 1)])
                        # drain PSUM via ACT so DVE top-k runs from SBUF and
                        # is decoupled from the PSUM slot ring
                        gcp = wk.tile([128, N], F16, tag="gcp", bufs=2,
                                      name=f"gcp{conv}_{g}_{t}")
                        nc.scalar.activation(gcp[:], ps[:], AF.Copy)
                        vals = wk.tile([128, 8], F16, tag="vals", bufs=2,
                                       name=f"vals{conv}_{g}_{t}")
                        nc.vector.max(out=vals[:], in_=gcp[:])
                        nc.vector.max_index(out=idxall[:, t, :], in_max=vals[:],
                                            in_values=gcp[:])
                    return th
                for t in range(NT):
                    thunks.append(mk(t))
                thunks.append(_wrap_thunk(g, idx_tag, 4 if conv == 1 else 8))
                return thunks

            def chunks_M1_setup(g):
                """Hoistable self-neighbor slab: pure matmuls from pos.

                Top-1 neighbor is always self (d=0), so its edge message is
                x_i @ W1a_top -- no gather or add; runs a window early.
                """
                def self_a1():
                    xr = S[g]["rhsA"][0:3, :]
                    pss = psp.tile([64, N], F32, tag="ps", name=f"pss{g}")
                    sig_pos = _sigma_read(xr)
                    for c in range(4):
                        nc.tensor.matmul(pss[:, 512 * c:512 * (c + 1)], W["w1aT"][:],
                                         sig_pos[:, 32 * c:32 * (c + 1), :])
                    r1a = wk.tile([64, N], F16, tag="r1a", bufs=2,
                                  name=f"r1a{g}_self")
                    nc.scalar.activation(r1a[:], pss[:], AF.Relu, bias=W["b1a"][:])
                    S[g]["r1a"] = r1a

                def self_a2():
                    r1a = S[g]["r1a"]
                    ps1b = psp.tile([64, N], F32, tag="ps", name=f"ps1bs{g}")
                    for c in range(4):
                        nc.tensor.matmul(ps1b[:, 512 * c:512 * (c + 1)], W["w1b"][:],
                                         r1a[:, 512 * c:512 * (c + 1)])
                    r1b = wk.tile([64, N], F16, tag="r1b", bufs=2,
                                  name=f"r1b{g}_self")
                    nc.scalar.activation(r1b[:], ps1b[:], AF.Relu, bias=W["b1b"][:])
                    S[g]["r1b"] = r1b

                def self_b():
                    r1b = S[g]["r1b"]
                    ps1c = psp.tile([64, N], F32, tag="ps", name=f"ps1cs{g}")
                    for c in range(4):
                        nc.tensor.matmul(ps1c[:, 512 * c:512 * (c + 1)], W["w1c"][:],
                                         r1b[:, 512 * c:512 * (c + 1)])
                    macc = wk.tile([64, N], F16, tag="macc", bufs=2,
                                   name=f"macc{g}")
                    S[g]["macc"] = macc
                    nc.scalar.activation(macc[:], ps1c[:], AF.Relu,
                                         bias=W["b1c"][:])

                return [self_a1, self_a2, self_b]

            def chunks_M1(g):
                ths = []

                def mk_slab_e(k):
                    def th():
                        posTmp = S[g]["posTmp"]
                        xr = S[g]["rhsA"][0:3, :]
                        gp = wk.tile([16, N], F32, tag="gath", bufs=2,
                                     name=f"gp{g}_{k}")
                        nc.gpsimd.ap_gather(
                            out_ap=gp[:].unsqueeze(-1),
                            in_ap=posTmp[:].unsqueeze(-1),
                            idxs_ap=S[g]["wrap1"][:, 128 * k:128 * (k + 1)],
                            channels=16, num_elems=N, d=1, num_idxs=N)
                        # edge layer a: A-part (x_i, sigma order) accumulated
                        # with B-part (gathered x_j) in PSUM -- no vector add
                        ps_e = psp.tile([64, N], F32, tag="ps",
                                        name=f"pse{g}_{k}")
                        sig_pos = _sigma_read(xr)
                        for c in range(4):
                            nc.tensor.matmul(ps_e[:, 512 * c:512 * (c + 1)],
                                             W["w1aA"][:],
                                             sig_pos[:, 32 * c:32 * (c + 1), :],
                                             start=True, stop=False)
                        for c in range(4):
                            nc.tensor.matmul(ps_e[:, 512 * c:512 * (c + 1)],
                                             W["w1aB"][:],
                                             gp[0:3, 512 * c:512 * (c + 1)],
                                             start=False, stop=True)
                        r1a = wk.tile([64, N], F16, tag="r1a", bufs=2,
                                      name=f"r1a{g}_{k}")
                        nc.scalar.activation(r1a[:], ps_e[:], AF.Relu,
                                             bias=W["b1a"][:])
                        S[g]["r1a"] = r1a
                    return th

                def mk_slab_b(k):
                    def th():
                        r1a = S[g]["r1a"]
                        ps1b = psp.tile([64, N], F32, tag="ps", name=f"ps1b{g}_{k}")
                        for c in range(4):
                            nc.tensor.matmul(ps1b[:, 512 * c:512 * (c + 1)],
                                             W["w1b"][:],
                                             r1a[:, 512 * c:512 * (c + 1)])
                        r1b = wk.tile([64, N], F16, tag="r1b", bufs=2,
                                      name=f"r1b{g}_{k}")
                        nc.scalar.activation(r1b[:], ps1b[:], AF.Relu,
                                             bias=W["b1b"][:])
                        S[g]["r1b"] = r1b
                    return th

                def mk_slab_c(k):
                    def th():
                        r1b = S[g]["r1b"]
                        ps1c = psp.tile([64, N], F32, tag="ps", name=f"ps1c{g}_{k}")
                        for c in range(4):
                            nc.tensor.matmul(ps1c[:, 512 * c:512 * (c + 1)],
                                             W["w1c"][:],
                                             r1b[:, 512 * c:512 * (c + 1)])
                        macc = S[g]["macc"]
                        r1c = wk.tile([64, N], F16, tag="r1c", bufs=2,
                                      name=f"r1c{g}_{k}")
                        nc.scalar.activation(r1c[:], ps1c[:], AF.Relu,
                                             bias=W["b1c"][:])
                        nc.vector.tensor_tensor(out=macc[:], in0=macc[:],
                                                in1=r1c[:], op=ALU.max)
                    return th

                for k in range(4):
                    ths.append(mk_slab_e(k))
                    ths.append(mk_slab_b(k))
                    ths.append(mk_slab_c(k))

                def x1_th():
                    macc = S[g]["macc"]
                    x1 = wk.tile([64, N], F16, tag="x1", bufs=2, name=f"x1_{g}")
                    nc.vector.tensor_copy(
                        x1[:].rearrange("c (b q) -> c b q", b=16, q=128),
                        macc[:].rearrange("c (q b) -> c b q", q=128, b=16))
                    S[g]["x1"] = x1
                ths.append(x1_th)
                return ths

            def chunks_B(g):
                def th():
                    x1 = S[g]["x1"]
                    sq2 = wk.tile([64, N], F16, tag="sq", bufs=1, name=f"sq2_{g}")
                    nc.scalar.activation(sq2[:], x1[:], AF.Square, scale=W["s1c"][:])
                    x2p2 = psp.tile([1, N], F32, tag="ps", name=f"x2p2_{g}")
                    for c in range(4):
                        nc.tensor.matmul(x2p2[:, 512 * c:512 * (c + 1)], ones64[:],
                                         sq2[:, 512 * c:512 * (c + 1)])
                    rhsA2 = wk.tile([65, N], F32R, tag="rhsA2", bufs=1,
                                    name=f"rhsA2_{g}")
                    nc.scalar.activation(rhsA2[0:64, :], x1[:], AF.Copy,
                                         scale=W["s1c"][:])
                    nc.scalar.activation(rhsA2[64:65, :], x2p2[:], AF.Copy,
                                         scale=-1.0)
                    lhsA2 = wk.tile([65, N], F32R, tag="lhsA2", bufs=1,
                                    name=f"lhsA2_{g}")
                    nc.scalar.activation(lhsA2[0:64, :], x1[:], AF.Copy,
                                         scale=W["s1c2"][:])
                    nc.sync.dma_start(lhsA2[64:65, :], S[g]["lhsA"][3:4, :])
                    S[g].update(rhsA2=rhsA2, lhsA2=lhsA2)
                return [th]

            def chunks_M2_setup(g):
                def setup_a():
                    x1 = S[g]["x1"]
                    psb2 = psp.tile([128, N], F32, tag="ps", name=f"psb2_{g}")
                    for c in range(4):
                        nc.tensor.matmul(psb2[:, 512 * c:512 * (c + 1)], W["w2B"][:],
                                         x1[:, 512 * c:512 * (c + 1)])
                    B2T = wk.tile([128, N], F32, tag="BT", bufs=2, name=f"B2T{g}")
                    nc.scalar.activation(B2T[:], psb2[:], AF.Copy)
                    S[g]["B2T"] = B2T

                def self_init():
                    # self-neighbor B-part: x1_i @ w2B' in sigma order
                    x1 = S[g]["x1"]
                    pss2 = psp.tile([128, N], F32, tag="ps", name=f"pss2_{g}")
                    sig_x1 = _sigma_read(x1[:])
                    for c in range(4):
                        nc.tensor.matmul(pss2[:, 512 * c:512 * (c + 1)], W["w2B"][:],
                                         sig_x1[:, 32 * c:32 * (c + 1), :])
                    macc2 = wk.tile([128, N], F32, tag="macc", bufs=2,
                                    name=f"macc2_{g}")
                    S[g]["macc2"] = macc2
                    nc.scalar.activation(macc2[:], pss2[:], AF.Copy)

                return [setup_a, self_init]

            def chunks_M2(g):
                ths = []

                def mk_gather(k):
                    def th():
                        B2T = S[g]["B2T"]
                        g2 = wk.tile([128, N], F32, tag="gath", bufs=2,
                                     name=f"g2_{g}_{k}")
                        nc.gpsimd.ap_gather(
                            out_ap=g2[:].unsqueeze(-1),
                            in_ap=B2T[:].unsqueeze(-1),
                            idxs_ap=S[g]["wrap2"][:, 128 * k:128 * (k + 1)],
                            channels=128, num_elems=N, d=1, num_idxs=N)
                        nc.vector.tensor_tensor(out=S[g]["macc2"][:],
                                                in0=S[g]["macc2"][:],
                                                in1=g2[:], op=ALU.max)
                    return th

                for k in range(4):
                    ths.append(mk_gather(k))

                def fin():
                    # x2 = relu(A-part(x_i) + max_k B-part + b2): A-part matmul
                    # accumulated with identity @ macc2 in PSUM
                    x1, macc2 = S[g]["x1"], S[g]["macc2"]
                    ps_f = psp.tile([128, N], F32, tag="ps", name=f"psf{g}")
                    sig_x1 = _sigma_read(x1[:])
                    for c in range(4):
                        nc.tensor.matmul(ps_f[:, 512 * c:512 * (c + 1)], W["w2A"][:],
                                         sig_x1[:, 32 * c:32 * (c + 1), :],
                                         start=True, stop=False)
                    for c in range(4):
                        nc.tensor.matmul(ps_f[:, 512 * c:512 * (c + 1)], ident[:],
                                         macc2[:, 512 * c:512 * (c + 1)],
                                         start=False, stop=True)
                    x2sg = wk.tile([128, N], F16, tag="x2sg", bufs=2,
                                   name=f"x2sg{g}")
                    nc.scalar.activation(x2sg[:], ps_f[:], AF.Relu, bias=W["b2"][:])
                    S[g]["x2sg"] = x2sg
                ths.append(fin)
                return ths

            def chunks_S15(g):
                def mk(mt):
                    def th():
                        x1, x2sg = S[g]["x1"], S[g]["x2sg"]
                        sig_x1 = _sigma_read(x1[:])
                        psl = psp.tile([128, N], F32, tag="ps", name=f"psl{g}_{mt}")
                        for c in range(4):
                            nc.tensor.matmul(psl[:, 512 * c:512 * (c + 1)],
                                             W["wl1"][:, 128 * mt:128 * (mt + 1)],
                                             sig_x1[:, 32 * c:32 * (c + 1), :],
                                             start=True, stop=False)
                        for c in range(4):
                            nc.tensor.matmul(psl[:, 512 * c:512 * (c + 1)],
                                             W["wl2"][:, 128 * mt:128 * (mt + 1)],
                                             x2sg[:, 512 * c:512 * (c + 1)],
                                             start=False, stop=True)
                        pr = wk.tile([128, 1], F32, tag="pr", bufs=2,
                                     name=f"pr{g}_{mt}")
                        nc.vector.tensor_reduce(pr[:], psl[:], axis=AX.X, op=ALU.max)
                        nc.scalar.activation(poolr[:, mt, g:g + 1], pr[:],
                                             AF.Relu, bias=W["bl"][:, mt:mt + 1])
                    return th
                return [mk(mt) for mt in range(8)]

            def zip_emit(tks, others):
                # even spread: emission position ~ readiness time, since PSUM
                # slot grants effectively follow emission priority
                n_t, n_o = len(tks), len(others)
                oi = 0
                for i, t in enumerate(tks):
                    t()
                    while oi < n_o and (oi + 1) * n_t <= (i + 1) * n_o:
                        others[oi]()
                        oi += 1
                while oi < n_o:
                    others[oi]()
                    oi += 1

            # ---------------- software-pipelined emission ----------------
            # A(g): pos load + gram1 operands; TK(g,c): gram+top-k conv c;
            # M1/M2: conv MLP (setup = B/A-part matmuls, hoisted one window
            # early); B(g): gram2 operands; S15(g): linear-l + global max pool.
            wload_all()
            for th in chunks_A(0):
                th()
            zip_emit(chunks_TK(0, 1), chunks_A(1) + chunks_M1_setup(0))
            zip_emit(chunks_TK(1, 1),
                     chunks_M1(0) + chunks_M1_setup(1) + chunks_B(0))
            zip_emit(chunks_TK(0, 2),
                     chunks_A(2) + chunks_M1(1) + chunks_M2_setup(0) + chunks_B(1))
            zip_emit(chunks_TK(2, 1),
                     chunks_M1_setup(2) + chunks_M2(0) + chunks_S15(0))
            zip_emit(chunks_TK(1, 2),
                     chunks_A(3) + chunks_M1(2) + chunks_M2_setup(1) + chunks_B(2))
            zip_emit(chunks_TK(3, 1),
                     chunks_M1_setup(3) + chunks_M2(1) + chunks_S15(1))
            zip_emit(chunks_TK(2, 2),
                     chunks_M1(3) + chunks_M2_setup(2) + chunks_B(3))
            zip_emit(chunks_TK(3, 2),
                     chunks_M2(2) + chunks_S15(2) + chunks_M2_setup(3))
            for th in chunks_M2(3) + chunks_S15(3):
                th()

            # ---------------- head MLP (all graphs) ----------------
            rm1 = pp.tile([128, 4, NG], F16, name="rm1")
            for mt in range(4):
                ph = psp.tile([128, NG], F32, tag="ps", name=f"ph1_{mt}")
                for kk in range(8):
                    nc.tensor.matmul(ph[:], W["wm1"][:, kk, 128 * mt:128 * (mt + 1)],
                                     poolr[:, kk, :], start=(kk == 0), stop=(kk == 7))
                nc.scalar.activation(rm1[:, mt, :], ph[:], AF.Relu,
                                     bias=W["bm1"][:, mt:mt + 1])
            rm2 = pp.tile([128, 2, NG], F16, name="rm2")
            for mt in range(2):
                ph = psp.tile([128, NG], F32, tag="ps", name=f"ph2_{mt}")
                for kk in range(4):
                    nc.tensor.matmul(ph[:], W["wm2"][:, kk, 128 * mt:128 * (mt + 1)],
                                     rm1[:, kk, :], start=(kk == 0), stop=(kk == 3))
                nc.scalar.activation(rm2[:, mt, :], ph[:], AF.Relu,
                                     bias=W["bm2"][:, mt:mt + 1])
            pho = psp.tile([2, NG], F32, tag="ps", name="pho")
            for kk in range(2):
                nc.tensor.matmul(pho[:], W["wout"][:, kk, :], rm2[:, kk, :],
                                 start=(kk == 0), stop=(kk == 1))
            outs = pp.tile([2, NG], F32, name="outs")
            nc.vector.tensor_scalar_add(outs[:], pho[:], W["bout"][:])
            nc.sync.dma_start(out_d[:], outs[:])

    nc.compile()
    return nc


def _fold_weights(inp):
    """Host-side BN folding / edge-weight splitting. Layout-only + tiny weight algebra."""
    f = {k: np.asarray(v, dtype=np.float64) for k, v in inp.items()}
    w = {}
    # conv1 layer a: e @ W1a = x_i @ (Wtop - Wbot) + x_j @ Wbot
    w["w1aA"] = (f["w1a"][:3] - f["w1a"][3:])
    w["w1aB"] = f["w1a"][3:]
    w["w1aT"] = f["w1a"][:3]
    w["b1a"] = f["b1a"]
    # fold (s1a, h1a) into layer b; (s1b, h1b) into layer c
    w["w1b"] = f["s1a"][:, None] * f["w1b"]
    w["b1b"] = f["h1a"] @ f["w1b"] + f["b1b"]
    w["w1c"] = f["s1b"][:, None] * f["w1c"]
    w["b1c"] = f["h1b"] @ f["w1c"] + f["b1c"]
    # (s1c, h1c) folded into all consumers of x1:
    w["s1c"] = f["s1c"]
    w["s1c2"] = 2.0 * f["s1c"]
    # conv2: e @ w2 = x_i @ (w2top - w2bot) + x_j @ w2bot, with x_i = s1c*x1 + h1c
    w["w2A"] = f["s1c"][:, None] * (f["w2"][:64] - f["w2"][64:])
    w["w2B"] = f["s1c"][:, None] * f["w2"][64:]
    w["b2"] = f["b2"] + f["h1c"] @ f["w2"][:64]
    # linear l: x1-part folded with (s1c, h1c); x2-part folded with (s2, h2)
    w["wl1"] = f["s1c"][:, None] * f["wl"][:64]
    w["wl2"] = f["s2"][:, None] * f["wl"][64:]
    w["bl"] = f["bl"] + f["h2"] @ f["wl"][64:] + f["h1c"] @ f["wl"][:64]
    # head: fold (sl, hl) into m1; (sm1, hm1) into m2; (sm2, hm2) into out
    w["wm1"] = f["sl"][:, None] * f["wm1"]
    w["bm1"] = f["hl"] @ f["wm1"] + f["bm1"]
    w["wm2"] = f["sm1"][:, None] * f["wm2"]
    w["bm2"] = f["hm1"] @ f["wm2"] + f["bm2"]
    w["wout"] = f["sm2"][:, None] * f["wout"]
    w["bout"] = f["hm2"] @ f["wout"] + f["bout"]
    return w


def _rep_mat(ngroups):
    """One-hot [64, 4, 16*ngroups]: rep[16k+b, k, 16g+b] = 1 (slots 1-4)."""
    r = np.zeros((64, 4, 16 * ngroups), dtype=np.float32)
    for k in range(4):
        for b in range(16):
            for g in range(ngroups):
                r[16 * k + b, k, 16 * g + b] = 1.0
    return r


def _weight_maps(w):
    f32 = lambda a: np.ascontiguousarray(a, dtype=np.float32)
    f16 = lambda a: np.ascontiguousarray(a, dtype=np.float16)
    m = {}
    m["w1aA"] = f32(w["w1aA"])
    m["w1aB"] = f32(w["w1aB"])
    m["w1aT"] = f32(w["w1aT"])
    m["w1b"] = f16(w["w1b"])
    m["w1c"] = f16(w["w1c"])
    m["w2A"] = f16(w["w2A"])
    m["w2B"] = f16(w["w2B"])
    m["wl1"] = f16(w["wl1"])
    m["wl2"] = f16(w["wl2"])
    m["wm1"] = f16(np.asarray(w["wm1"]).reshape(8, 128, 512).transpose(1, 0, 2))
    m["wm2"] = f16(np.asarray(w["wm2"]).reshape(4, 128, 256).transpose(1, 0, 2))
    m["wout"] = f16(np.asarray(w["wout"]).reshape(2, 128, 2).transpose(1, 0, 2))
    m["rep4"] = _rep_mat(4)
    m["rep8"] = _rep_mat(8)
    m["b1a"] = f32(w["b1a"].reshape(64, 1))
    m["b1b"] = f32(w["b1b"].reshape(64, 1))
    m["b1c"] = f32(w["b1c"].reshape(64, 1))
    m["s1c"] = f32(w["s1c"].reshape(64, 1))
    m["s1c2"] = f32(w["s1c2"].reshape(64, 1))
    m["b2"] = f32(w["b2"].reshape(128, 1))
    m["bl"] = f32(np.asarray(w["bl"]).reshape(8, 128).T)
    m["bm1"] = f32(np.asarray(w["bm1"]).reshape(4, 128).T)
    m["bm2"] = f32(np.asarray(w["bm2"]).reshape(2, 128).T)
    m["bout"] = f32(w["bout"].reshape(2, 1))
    return m


def _build_in_maps(inputs):
    w = _fold_weights(inputs)
    wm = _weight_maps(w)
    pos64 = np.asarray(inputs["pos"], dtype=np.float64)  # [B, N, 3]
    B = pos64.shape[0]
    posT = np.zeros((B, 36, pos64.shape[1]), dtype=np.float32)
    xt = pos64.transpose(0, 2, 1)                        # [B, 3, N]
    posT[:, 0:3] = xt
    posT[:, 3] = -np.sum(pos64 * pos64, axis=2)          # -|x|^2
    posT[:, 32:35] = xt
    posT[:, 35] = 0.5
    in_maps = []
    for c in range(NCORES):
        m = dict(wm)
        m["posT"] = np.ascontiguousarray(posT[NG * c:NG * (c + 1)])
        in_maps.append(m)
    return in_maps


def kernel(**inputs):
    if "nc" not in _CACHE:
        _CACHE["nc"] = build_nc()
    nc = _CACHE["nc"]

    in_maps = _build_in_maps(inputs)
    B = np.asarray(inputs["pos"]).shape[0]

    res = bass_utils.run_bass_kernel_spmd(nc, in_maps, core_ids=list(range(NCORES)))
    out = np.zeros((B, 2), dtype=np.float32)
    for c in range(NCORES):
        out[NG * c:NG * (c + 1)] = res.results[c]["out"].T
    return out


# revision 37
# speedup vs baseline: 1.1552x; 1.1552x over previous
"""DGCNN (2x EdgeConv kNN=5 + MLP head) Trainium2 kernel, data-parallel over 8 NeuronCores.

Contract: kernel(**inputs) takes the FULL inputs of nn_DEC_41180146434796
(pos [32,2048,3] + MLP weights) and returns the FULL [32,2] output.
Each core processes 4 graphs end-to-end (kNN, gathers, max-aggregations local).

v2 design notes:
- The DVE top-k (max8 + find_index8 over [128,2048] gram tiles) is the hard
  floor (~75us per conv-graph). Everything else is moved off the DVE
  (conv adds/maxes -> Pool, relu/casts -> ACT, BN folds -> weights) and the
  4 graphs are software-pipelined: each graph's conv/MLP phase is emitted
  fine-grained-interleaved into the NEXT top-k phase's gram tiles so PSUM
  ring slot priority matches the intended overlap.
- Gram operands drop the per-row-constant |x_i|^2 (rank-preserving):
  lhs = [2x; 1], rhs = [x; -|x_j|^2].
- find_index8 results are redistributed to the gather's wrapped layout with
  a PE transpose + one-hot replication matmuls (no serial DMA storm).
"""
import numpy as np

import concourse.bass as bass
import concourse.mybir as mybir
from concourse import bacc, tile
from concourse import bass_utils
from concourse.masks import make_identity

F32 = mybir.dt.float32
F32R = mybir.dt.float32r
F16 = mybir.dt.float16
U32 = mybir.dt.uint32
I16 = mybir.dt.int16
AF = mybir.ActivationFunctionType
ALU = mybir.AluOpType
AX = mybir.AxisListType

N = 2048          # nodes per graph
NG = 4            # graphs per core
K = 5             # kNN neighbors (incl self)
NT = 16           # node tiles of 128
NCORES = 8

_CACHE = {}


def _sigma_read(ap):
    """View a [C, 2048] natural-ordered tensor so its free stream is sigma-ordered.

    sigma col s = 16*q + b  <->  node i = 128*b + q.
    """
    return ap.rearrange("c (b q) -> c q b", b=16, q=128)


def build_nc():
    nc = bacc.Bacc(None, target_bir_lowering=False)

    # ---------------- I/O ----------------
    posT_d = nc.dram_tensor("posT", [NG, 20, N], F32, kind="ExternalInput")
    w1aA_d = nc.dram_tensor("w1aA", [3, 64], F32, kind="ExternalInput")
    w1aB_d = nc.dram_tensor("w1aB", [3, 64], F32, kind="ExternalInput")
    w1b_d = nc.dram_tensor("w1b", [64, 64], F16, kind="ExternalInput")
    w1c_d = nc.dram_tensor("w1c", [64, 64], F16, kind="ExternalInput")
    w2A_d = nc.dram_tensor("w2A", [64, 128], F16, kind="ExternalInput")
    w2B_d = nc.dram_tensor("w2B", [64, 128], F16, kind="ExternalInput")
    wl1_d = nc.dram_tensor("wl1", [64, 1024], F16, kind="ExternalInput")
    wl2_d = nc.dram_tensor("wl2", [128, 1024], F16, kind="ExternalInput")
    wm1_d = nc.dram_tensor("wm1", [128, 8, 512], F16, kind="ExternalInput")
    wm2_d = nc.dram_tensor("wm2", [128, 4, 256], F16, kind="ExternalInput")
    wout_d = nc.dram_tensor("wout", [128, 2, 2], F16, kind="ExternalInput")
    rep4_d = nc.dram_tensor("rep4", [64, 4, 64], F32, kind="ExternalInput")
    rep8_d = nc.dram_tensor("rep8", [64, 4, 128], F32, kind="ExternalInput")
    w1aT_d = nc.dram_tensor("w1aT", [3, 64], F32, kind="ExternalInput")
    b1a_d = nc.dram_tensor("b1a", [64, 1], F32, kind="ExternalInput")
    b1b_d = nc.dram_tensor("b1b", [64, 1], F32, kind="ExternalInput")
    b1c_d = nc.dram_tensor("b1c", [64, 1], F32, kind="ExternalInput")
    s1c_d = nc.dram_tensor("s1c", [64, 1], F32, kind="ExternalInput")
    s1c2_d = nc.dram_tensor("s1c2", [64, 1], F32, kind="ExternalInput")
    b2_d = nc.dram_tensor("b2", [128, 1], F32, kind="ExternalInput")
    bl_d = nc.dram_tensor("bl", [128, 8], F32, kind="ExternalInput")
    bm1_d = nc.dram_tensor("bm1", [128, 4], F32, kind="ExternalInput")
    bm2_d = nc.dram_tensor("bm2", [128, 2], F32, kind="ExternalInput")
    bout_d = nc.dram_tensor("bout", [2, 1], F32, kind="ExternalInput")

    out_d = nc.dram_tensor("out", [2, NG], F32, kind="ExternalOutput")

    with tile.TileContext(nc) as tc:
        with tc.tile_pool(name="wpool", bufs=1) as wp, \
             tc.tile_pool(name="persist", bufs=1) as pp, \
             tc.tile_pool(name="work", bufs=1) as wk, \
             tc.tile_pool(name="ps", bufs=2, space="PSUM") as psp:

            W = {}

            def wload_all():
                def wload(dram, shape, dtype, name):
                    t = wp.tile(shape, dtype, name=name)
                    if dtype == F32R:
                        nc.sync.dma_start(t[:].bitcast(F32), dram[:])
                    else:
                        nc.sync.dma_start(t[:], dram[:])
                    return t

                def wload_r(dram, shape, name):
                    tmp = wk.tile(shape, F32, tag="idxf", bufs=2, name=name + "_stg")
                    nc.sync.dma_start(tmp[:], dram[:])
                    t = wp.tile(shape, F32R, name=name)
                    nc.scalar.activation(t[:], tmp[:], AF.Copy)
                    return t

                W["w1aA"] = wload_r(w1aA_d, [3, 64], "w1aA_s")
                W["w1aB"] = wload(w1aB_d, [3, 64], F32, "w1aB_s")
                W["w1b"] = wload(w1b_d, [64, 64], F16, "w1b_s")
                W["w1c"] = wload(w1c_d, [64, 64], F16, "w1c_s")
                W["w2A"] = wload(w2A_d, [64, 128], F16, "w2A_s")
                W["w2B"] = wload(w2B_d, [64, 128], F16, "w2B_s")
                W["wl1"] = wload(wl1_d, [64, 1024], F16, "wl1_s")
                W["wl2"] = wload(wl2_d, [128, 1024], F16, "wl2_s")
                W["wm1"] = wload(wm1_d, [128, 8, 512], F16, "wm1_s")
                W["wm2"] = wload(wm2_d, [128, 4, 256], F16, "wm2_s")
                W["wout"] = wload(wout_d, [128, 2, 2], F16, "wout_s")
                W["rep4"] = wload(rep4_d, [64, 4, 64], F32, "rep4_s")
                W["rep8"] = wload(rep8_d, [64, 4, 128], F32, "rep8_s")
                W["w1aT"] = wload_r(w1aT_d, [3, 64], "w1aT_s")
                W["b1a"] = wload(b1a_d, [64, 1], F32, "b1a_s")
                W["b1b"] = wload(b1b_d, [64, 1], F32, "b1b_s")
                W["b1c"] = wload(b1c_d, [64, 1], F32, "b1c_s")
                W["s1c"] = wload(s1c_d, [64, 1], F32, "s1c_s")
                W["s1c2"] = wload(s1c2_d, [64, 1], F32, "s1c2_s")
                W["b2"] = wload(b2_d, [128, 1], F32, "b2_s")
                W["bl"] = wload(bl_d, [128, 8], F32, "bl_s")
                W["bm1"] = wload(bm1_d, [128, 4], F32, "bm1_s")
                W["bm2"] = wload(bm2_d, [128, 2], F32, "bm2_s")
                W["bout"] = wload(bout_d, [2, 1], F32, "bout_s")

            ident = wp.tile([128, 128], F32, name="ident")
            make_identity(nc, ident[:])
            ones64 = wp.tile([64, 1], F16, name="ones64")
            nc.vector.memset(ones64[:], 1.0)

            # pooled & relu'd features for the head: [128, mt(8), graph(4)]
            poolr = pp.tile([128, 8, NG], F16, name="poolr")

            S = [dict() for _ in range(NG)]

            # ---------------- stage chunk builders ----------------
            def chunks_A(g):
                def th():
                    posTmp = wk.tile([36, N], F32, tag="posT", bufs=2,
                                     name=f"posTmp{g}")
                    nc.sync.dma_start(posTmp[0:16, :], posT_d[g, 0:16])
                    nc.sync.dma_start(posTmp[32:36, :], posT_d[g, 16:20])
                    # host rows: [x; -|x|^2] at 0-3 (also the 16-ch gather
                    # source), [x; 0.5] at 32-35 (engine reads must start at
                    # 0/32/64); ACT copies round to f32r for the gram matmuls
                    rhsA = wk.tile([4, N], F32R, tag="rhsA", bufs=2,
                                   name=f"rhsA{g}")
                    nc.scalar.activation(rhsA[:], posTmp[0:4, :], AF.Copy)
                    lhsA = wk.tile([4, N], F32R, tag="lhsA", bufs=2,
                                   name=f"lhsA{g}")
                    nc.scalar.activation(lhsA[:], posTmp[32:36, :], AF.Copy,
                                         scale=2.0)
                    S[g].update(posTmp=posTmp, rhsA=rhsA, lhsA=lhsA)
                return [th]

            def _wrap_thunk(g, idx_tag, ngroups):
                def th():
                    idxall = S[g][idx_tag]
                    idxf = wk.tile([128, 4, 16], F32, tag="idxf", bufs=2,
                                   name=f"idxf_{idx_tag}{g}")
                    nc.vector.tensor_copy(idxf[:],
                                          idxall[:, :, 1:5].transpose([0, 2, 1]))
                    tp = psp.tile([64, 128], F32, tag="ps", name=f"tp{idx_tag}{g}")
                    nc.tensor.transpose(tp[:], idxf[:].rearrange("p a b -> p (a b)"),
                                        ident[:])
                    idxtf = wk.tile([64, 128], F32, tag="idxtf", bufs=2,
                                    name=f"idxtf_{idx_tag}{g}")
                    nc.scalar.activation(idxtf[:], tp[:], AF.Copy)
                    rep = W["rep4"] if ngroups == 4 else W["rep8"]
                    wrapP = psp.tile([16 * ngroups, 512], F32, tag="ps",
                                     name=f"wrapP{idx_tag}{g}")
                    for k in range(4):
                        nc.tensor.matmul(wrapP[:, 128 * k:128 * (k + 1)],
                                         rep[:, k, :], idxtf[:])
                    wrap = wk.tile([16 * ngroups, 512], I16, tag=f"wrap{ngroups}",
                                   bufs=2, name=f"wrap{ngroups}_{g}")
                    nc.scalar.activation(wrap[:], wrapP[:], AF.Copy)
                    S[g][f"wrap{1 if ngroups == 4 else 2}"] = wrap
                return th

            def chunks_TK(g, conv):
                lhs_key = "lhsA" if conv == 1 else "lhsA2"
                rhs_key = "rhsA" if conv == 1 else "rhsA2"
                idx_tag = f"idx{conv}"
                thunks = []
                # idxall tile created in first chunk
                def mk(t):
                    def th():
                        if t == 0:
                            S[g][idx_tag] = wk.tile([128, NT, 8], U32, tag=idx_tag,
                                                    bufs=2, name=f"{idx_tag}_{g}")
                        idxall = S[g][idx_tag]
                        lhs, rhs = S[g][lhs_key], S[g][rhs_key]
                        ps = psp.tile([128, N], F32, tag="ps",
                                      name=f"gps{conv}_{g}_{t}")
                        for c in range(4):
                            nc.tensor.matmul(ps[:, 512 * c:512 * (c + 1)],
                                             lhs[:, 128 * t:128 * (t + 1)],
                                             rhs[:, 512 * c:512 * (c + 1)])
                        # drain PSUM via ACT so DVE top-k runs from SBUF and
                        # is decoupled from the PSUM slot ring
                        gcp = wk.tile([128, N], F32, tag="gcp", bufs=2,
                                      name=f"gcp{conv}_{g}_{t}")
                        nc.scalar.activation(gcp[:], ps[:], AF.Copy)
                        vals = wk.tile([128, 8], F32, tag="vals", bufs=2,
                                       name=f"vals{conv}_{g}_{t}")
                        nc.vector.max(out=vals[:], in_=gcp[:])
                        nc.vector.max_index(out=idxall[:, t, :], in_max=vals[:],
                                            in_values=gcp[:])
                    return th
                for t in range(NT):
                    thunks.append(mk(t))
                thunks.append(_wrap_thunk(g, idx_tag, 4 if conv == 1 else 8))
                return thunks

            def chunks_M1_setup(g):
                """Hoistable self-neighbor slab: pure matmuls from pos.

                Top-1 neighbor is always self (d=0), so its edge message is
                x_i @ W1a_top -- no gather or add; runs a window early.
                """
                def self_a1():
                    xr = S[g]["rhsA"][0:3, :]
                    pss = psp.tile([64, N], F32, tag="ps", name=f"pss{g}")
                    sig_pos = _sigma_read(xr)
                    for c in range(4):
                        nc.tensor.matmul(pss[:, 512 * c:512 * (c + 1)], W["w1aT"][:],
                                         sig_pos[:, 32 * c:32 * (c + 1), :])
                    r1a = wk.tile([64, N], F16, tag="r1a", bufs=2,
                                  name=f"r1a{g}_self")
                    nc.scalar.activation(r1a[:], pss[:], AF.Relu, bias=W["b1a"][:])
                    S[g]["r1a"] = r1a

                def self_a2():
                    r1a = S[g]["r1a"]
                    ps1b = psp.tile([64, N], F32, tag="ps", name=f"ps1bs{g}")
                    for c in range(4):
                        nc.tensor.matmul(ps1b[:, 512 * c:512 * (c + 1)], W["w1b"][:],
                                         r1a[:, 512 * c:512 * (c + 1)])
                    r1b = wk.tile([64, N], F16, tag="r1b", bufs=2,
                                  name=f"r1b{g}_self")
                    nc.scalar.activation(r1b[:], ps1b[:], AF.Relu, bias=W["b1b"][:])
                    S[g]["r1b"] = r1b

                def self_b():
                    r1b = S[g]["r1b"]
                    ps1c = psp.tile([64, N], F32, tag="ps", name=f"ps1cs{g}")
                    for c in range(4):
                        nc.tensor.matmul(ps1c[:, 512 * c:512 * (c + 1)], W["w1c"][:],
                                         r1b[:, 512 * c:512 * (c + 1)])
                    macc = wk.tile([64, N], F16, tag="macc", bufs=2,
                                   name=f"macc{g}")
                    S[g]["macc"] = macc
                    nc.scalar.activation(macc[:], ps1c[:], AF.Relu,
                                         bias=W["b1c"][:])

                return [self_a1, self_a2, self_b]

            def chunks_M1(g):
                ths = []

                def mk_slab_e(k):
                    def th():
                        posTmp = S[g]["posTmp"]
                        xr = S[g]["rhsA"][0:3, :]
                        gp = wk.tile([16, N], F32, tag="gath", bufs=3,
                                     name=f"gp{g}_{k}")
                        nc.gpsimd.ap_gather(
                            out_ap=gp[:].unsqueeze(-1),
                            in_ap=posTmp[:].unsqueeze(-1),
                            idxs_ap=S[g]["wrap1"][:, 128 * k:128 * (k + 1)],
                            channels=16, num_elems=N, d=1, num_idxs=N)
                        # edge layer a: A-part (x_i, sigma order) accumulated
                        # with B-part (gathered x_j) in PSUM -- no vector add
                        ps_e = psp.tile([64, N], F32, tag="ps",
                                        name=f"pse{g}_{k}")
                        sig_pos = _sigma_read(xr)
                        for c in range(4):
                            nc.tensor.matmul(ps_e[:, 512 * c:512 * (c + 1)],
                                             W["w1aA"][:],
                                             sig_pos[:, 32 * c:32 * (c + 1), :],
                                             start=True, stop=False)
                        for c in range(4):
                            nc.tensor.matmul(ps_e[:, 512 * c:512 * (c + 1)],
                                             W["w1aB"][:],
                                             gp[0:3, 512 * c:512 * (c + 1)],
                                             start=False, stop=True)
                        r1a = wk.tile([64, N], F16, tag="r1a", bufs=2,
                                      name=f"r1a{g}_{k}")
                        nc.scalar.activation(r1a[:], ps_e[:], AF.Relu,
                                             bias=W["b1a"][:])
                        S[g]["r1a"] = r1a
                    return th

                def mk_slab_b(k):
                    def th():
                        r1a = S[g]["r1a"]
                        ps1b = psp.tile([64, N], F32, tag="ps", name=f"ps1b{g}_{k}")
                        for c in range(4):
                            nc.tensor.matmul(ps1b[:, 512 * c:512 * (c + 1)],
                                             W["w1b"][:],
                                             r1a[:, 512 * c:512 * (c + 1)])
                        r1b = wk.tile([64, N], F16, tag="r1b", bufs=2,
                                      name=f"r1b{g}_{k}")
                        nc.scalar.activation(r1b[:], ps1b[:], AF.Relu,
                                             bias=W["b1b"][:])
                        S[g]["r1b"] = r1b
                    return th

                def mk_slab_c(k):
                    def th():
                        r1b = S[g]["r1b"]
                        ps1c = psp.tile([64, N], F32, tag="ps", name=f"ps1c{g}_{k}")
                        for c in range(4):
                            nc.tensor.matmul(ps1c[:, 512 * c:512 * (c + 1)],
                                             W["w1c"][:],
                                             r1b[:, 512 * c:512 * (c + 1)])
                        macc = S[g]["macc"]
                        r1c = wk.tile([64, N], F16, tag="r1c", bufs=2,
                                      name=f"r1c{g}_{k}")
                        nc.scalar.activation(r1c[:], ps1c[:], AF.Relu,
                                             bias=W["b1c"][:])
                        nc.vector.tensor_tensor(out=macc[:], in0=macc[:],
                                                in1=r1c[:], op=ALU.max)
                    return th

                for k in range(4):
                    ths.append(mk_slab_e(k))
                    ths.append(mk_slab_b(k))
                    ths.append(mk_slab_c(k))

                def x1_th():
                    macc = S[g]["macc"]
                    x1 = wk.tile([64, N], F16, tag="x1", bufs=2, name=f"x1_{g}")
                    nc.scalar.activation(
                        x1[:].rearrange("c (b q) -> c b q", b=16, q=128),
                        macc[:].rearrange("c (q b) -> c b q", q=128, b=16),
                        AF.Copy)
                    S[g]["x1"] = x1
                ths.append(x1_th)
                return ths

            def chunks_B(g):
                def th():
                    x1 = S[g]["x1"]
                    sq2 = wk.tile([64, N], F16, tag="sq", bufs=1, name=f"sq2_{g}")
                    nc.scalar.activation(sq2[:], x1[:], AF.Square, scale=W["s1c"][:])
                    x2p2 = psp.tile([1, N], F32, tag="ps", name=f"x2p2_{g}")
                    for c in range(4):
                        nc.tensor.matmul(x2p2[:, 512 * c:512 * (c + 1)], ones64[:],
                                         sq2[:, 512 * c:512 * (c + 1)])
                    rhsA2 = wk.tile([65, N], F32R, tag="rhsA2", bufs=1,
                                    name=f"rhsA2_{g}")
                    nc.scalar.activation(rhsA2[0:64, :], x1[:], AF.Copy,
                                         scale=W["s1c"][:])
                    nc.scalar.activation(rhsA2[64:65, :], x2p2[:], AF.Copy,
                                         scale=-1.0)
                    lhsA2 = wk.tile([65, N], F32R, tag="lhsA2", bufs=1,
                                    name=f"lhsA2_{g}")
                    nc.scalar.activation(lhsA2[0:64, :], x1[:], AF.Copy,
                                         scale=W["s1c2"][:])
                    nc.sync.dma_start(lhsA2[64:65, :], S[g]["lhsA"][3:4, :])
                    S[g].update(rhsA2=rhsA2, lhsA2=lhsA2)
                return [th]

            def chunks_M2_setup(g):
                def setup_a():
                    x1 = S[g]["x1"]
                    psb2 = psp.tile([128, N], F32, tag="ps", name=f"psb2_{g}")
                    for c in range(4):
                        nc.tensor.matmul(psb2[:, 512 * c:512 * (c + 1)], W["w2B"][:],
                                         x1[:, 512 * c:512 * (c + 1)])
                    B2T = wk.tile([128, N], F32, tag="BT", bufs=2, name=f"B2T{g}")
                    nc.scalar.activation(B2T[:], psb2[:], AF.Copy)
                    S[g]["B2T"] = B2T

                def self_init():
                    # self-neighbor B-part: x1_i @ w2B' in sigma order
                    x1 = S[g]["x1"]
                    pss2 = psp.tile([128, N], F32, tag="ps", name=f"pss2_{g}")
                    sig_x1 = _sigma_read(x1[:])
                    for c in range(4):
                        nc.tensor.matmul(pss2[:, 512 * c:512 * (c + 1)], W["w2B"][:],
                                         sig_x1[:, 32 * c:32 * (c + 1), :])
                    macc2 = wk.tile([128, N], F32, tag="macc", bufs=2,
                                    name=f"macc2_{g}")
                    S[g]["macc2"] = macc2
                    nc.scalar.activation(macc2[:], pss2[:], AF.Copy)

                return [setup_a, self_init]

            def chunks_M2(g):
                # gathers lead their DVE maxes by 2 chunks so the static DVE
                # stream never waits on the Pool
                def mk_gather(k):
                    def th():
                        B2T = S[g]["B2T"]
                        g2 = wk.tile([128, N], F32, tag="gath", bufs=3,
                                     name=f"g2_{g}_{k}")
                        S[g][f"g2_{k}"] = g2
                        nc.gpsimd.ap_gather(
                            out_ap=g2[:].unsqueeze(-1),
                            in_ap=B2T[:].unsqueeze(-1),
                            idxs_ap=S[g]["wrap2"][:, 128 * k:128 * (k + 1)],
                            channels=128, num_elems=N, d=1, num_idxs=N)
                    return th

                def mk_max(k):
                    def th():
                        nc.vector.tensor_tensor(out=S[g]["macc2"][:],
                                                in0=S[g]["macc2"][:],
                                                in1=S[g][f"g2_{k}"][:], op=ALU.max)
                    return th

                def fin():
                    # x2 = relu(A-part(x_i) + max_k B-part + b2): A-part matmul
                    # accumulated with identity @ macc2 in PSUM
                    x1, macc2 = S[g]["x1"], S[g]["macc2"]
                    ps_f = psp.tile([128, N], F32, tag="ps", name=f"psf{g}")
                    sig_x1 = _sigma_read(x1[:])
                    for c in range(4):
                        nc.tensor.matmul(ps_f[:, 512 * c:512 * (c + 1)], W["w2A"][:],
                                         sig_x1[:, 32 * c:32 * (c + 1), :],
                                         start=True, stop=False)
                    for c in range(4):
                        nc.tensor.matmul(ps_f[:, 512 * c:512 * (c + 1)], ident[:],
                                         macc2[:, 512 * c:512 * (c + 1)],
                                         start=False, stop=True)
                    x2sg = wk.tile([128, N], F16, tag="x2sg", bufs=2,
                                   name=f"x2sg{g}")
                    nc.scalar.activation(x2sg[:], ps_f[:], AF.Relu, bias=W["b2"][:])
                    S[g]["x2sg"] = x2sg

                ths = [mk_gather(0), mk_gather(1),
                       lambda: (mk_gather(2)(), mk_max(0)()),
                       lambda: (mk_gather(3)(), mk_max(1)()),
                       mk_max(2), mk_max(3), fin]
                return ths

            def chunks_S15(g):
                def mk(mt):
                    def th():
                        x1, x2sg = S[g]["x1"], S[g]["x2sg"]
                        sig_x1 = _sigma_read(x1[:])
                        psl = psp.tile([128, N], F32, tag="ps", name=f"psl{g}_{mt}")
                        for c in range(4):
                            nc.tensor.matmul(psl[:, 512 * c:512 * (c + 1)],
                                             W["wl1"][:, 128 * mt:128 * (mt + 1)],
                                             sig_x1[:, 32 * c:32 * (c + 1), :],
                                             start=True, stop=False)
                        for c in range(4):
                            nc.tensor.matmul(psl[:, 512 * c:512 * (c + 1)],
                                             W["wl2"][:, 128 * mt:128 * (mt + 1)],
                                             x2sg[:, 512 * c:512 * (c + 1)],
                                             start=False, stop=True)
                        pr = wk.tile([128, 1], F32, tag="pr", bufs=2,
                                     name=f"pr{g}_{mt}")
                        nc.vector.tensor_reduce(pr[:], psl[:], axis=AX.X, op=ALU.max)
                        nc.scalar.activation(poolr[:, mt, g:g + 1], pr[:],
                                             AF.Relu, bias=W["bl"][:, mt:mt + 1])
                    return th
                return [mk(mt) for mt in range(8)]

            def zip_emit(tks, others):
                # even spread: emission position ~ readiness time, since PSUM
                # slot grants effectively follow emission priority
                n_t, n_o = len(tks), len(others)
                oi = 0
                for i, t in enumerate(tks):
                    t()
                    while oi < n_o and (oi + 1) * n_t <= (i + 1) * n_o:
                        others[oi]()
                        oi += 1
                while oi < n_o:
                    others[oi]()
                    oi += 1

            # ---------------- software-pipelined emission ----------------
            # A(g): pos load + gram1 operands; TK(g,c): gram+top-k conv c;
            # M1/M2: conv MLP (setup = B/A-part matmuls, hoisted one window
            # early); B(g): gram2 operands; S15(g): linear-l + global max pool.
            for th in chunks_A(0):
                th()
            wload_all()
            zip_emit(chunks_TK(0, 1), chunks_A(1) + chunks_M1_setup(0))
            zip_emit(chunks_TK(1, 1),
                     chunks_M1(0) + chunks_M1_setup(1) + chunks_B(0))
            zip_emit(chunks_TK(0, 2),
                     chunks_A(2) + chunks_M1(1) + chunks_M2_setup(0) + chunks_B(1))
            zip_emit(chunks_TK(2, 1),
                     chunks_M1_setup(2) + chunks_M2(0) + chunks_S15(0))
            zip_emit(chunks_TK(1, 2),
                     chunks_A(3) + chunks_M1(2) + chunks_M2_setup(1) + chunks_B(2))
            zip_emit(chunks_TK(3, 1),
                     chunks_M1_setup(3) + chunks_M2(1) + chunks_S15(1))
            zip_emit(chunks_TK(2, 2),
                     chunks_M1(3) + chunks_M2_setup(2) + chunks_B(3))
            zip_emit(chunks_TK(3, 2),
                     chunks_M2(2) + chunks_S15(2) + chunks_M2_setup(3))
            for th in chunks_M2(3) + chunks_S15(3):
                th()

            # ---------------- head MLP (all graphs) ----------------
            rm1 = pp.tile([128, 4, NG], F16, name="rm1")
            for mt in range(4):
                ph = psp.tile([128, NG], F32, tag="ps", name=f"ph1_{mt}")
                for kk in range(8):
                    nc.tensor.matmul(ph[:], W["wm1"][:, kk, 128 * mt:128 * (mt + 1)],
                                     poolr[:, kk, :], start=(kk == 0), stop=(kk == 7))
                nc.scalar.activation(rm1[:, mt, :], ph[:], AF.Relu,
                                     bias=W["bm1"][:, mt:mt + 1])
            rm2 = pp.tile([128, 2, NG], F16, name="rm2")
            for mt in range(2):
                ph = psp.tile([128, NG], F32, tag="ps", name=f"ph2_{mt}")
                for kk in range(4):
                    nc.tensor.matmul(ph[:], W["wm2"][:, kk, 128 * mt:128 * (mt + 1)],
                                     rm1[:, kk, :], start=(kk == 0), stop=(kk == 3))
                nc.scalar.activation(rm2[:, mt, :], ph[:], AF.Relu,
                                     bias=W["bm2"][:, mt:mt + 1])
            pho = psp.tile([2, NG], F32, tag="ps", name="pho")
            for kk in range(2):
                nc.tensor.matmul(pho[:], W["wout"][:, kk, :], rm2[:, kk, :],
                                 start=(kk == 0), stop=(kk == 1))
            outs = pp.tile([2, NG], F32, name="outs")
            nc.vector.tensor_scalar_add(outs[:], pho[:], W["bout"][:])
            nc.sync.dma_start(out_d[:], outs[:])

    nc.compile()
    return nc


def _fold_weights(inp):
    """Host-side BN folding / edge-weight splitting. Layout-only + tiny weight algebra."""
    f = {k: np.asarray(v, dtype=np.float64) for k, v in inp.items()}
    w = {}
    # conv1 layer a: e @ W1a = x_i @ (Wtop - Wbot) + x_j @ Wbot
    w["w1aA"] = (f["w1a"][:3] - f["w1a"][3:])
    w["w1aB"] = f["w1a"][3:]
    w["w1aT"] = f["w1a"][:3]
    w["b1a"] = f["b1a"]
    # fold (s1a, h1a) into layer b; (s1b, h1b) into layer c
    w["w1b"] = f["s1a"][:, None] * f["w1b"]
    w["b1b"] = f["h1a"] @ f["w1b"] + f["b1b"]
    w["w1c"] = f["s1b"][:, None] * f["w1c"]
    w["b1c"] = f["h1b"] @ f["w1c"] + f["b1c"]
    # (s1c, h1c) folded into all consumers of x1:
    w["s1c"] = f["s1c"]
    w["s1c2"] = 2.0 * f["s1c"]
    # conv2: e @ w2 = x_i @ (w2top - w2bot) + x_j @ w2bot, with x_i = s1c*x1 + h1c
    w["w2A"] = f["s1c"][:, None] * (f["w2"][:64] - f["w2"][64:])
    w["w2B"] = f["s1c"][:, None] * f["w2"][64:]
    w["b2"] = f["b2"] + f["h1c"] @ f["w2"][:64]
    # linear l: x1-part folded with (s1c, h1c); x2-part folded with (s2, h2)
    w["wl1"] = f["s1c"][:, None] * f["wl"][:64]
    w["wl2"] = f["s2"][:, None] * f["wl"][64:]
    w["bl"] = f["bl"] + f["h2"] @ f["wl"][64:] + f["h1c"] @ f["wl"][:64]
    # head: fold (sl, hl) into m1; (sm1, hm1) into m2; (sm2, hm2) into out
    w["wm1"] = f["sl"][:, None] * f["wm1"]
    w["bm1"] = f["hl"] @ f["wm1"] + f["bm1"]
    w["wm2"] = f["sm1"][:, None] * f["wm2"]
    w["bm2"] = f["hm1"] @ f["wm2"] + f["bm2"]
    w["wout"] = f["sm2"][:, None] * f["wout"]
    w["bout"] = f["hm2"] @ f["wout"] + f["bout"]
    return w


def _rep_mat(ngroups):
    """One-hot [64, 4, 16*ngroups]: rep[16k+b, k, 16g+b] = 1 (slots 1-4)."""
    r = np.zeros((64, 4, 16 * ngroups), dtype=np.float32)
    for k in range(4):
        for b in range(16):
            for g in range(ngroups):
                r[16 * k + b, k, 16 * g + b] = 1.0
    return r


def _weight_maps(w):
    f32 = lambda a: np.ascontiguousarray(a, dtype=np.float32)
    f16 = lambda a: np.ascontiguousarray(a, dtype=np.float16)
    m = {}
    m["w1aA"] = f32(w["w1aA"])
    m["w1aB"] = f32(w["w1aB"])
    m["w1aT"] = f32(w["w1aT"])
    m["w1b"] = f16(w["w1b"])
    m["w1c"] = f16(w["w1c"])
    m["w2A"] = f16(w["w2A"])
    m["w2B"] = f16(w["w2B"])
    m["wl1"] = f16(w["wl1"])
    m["wl2"] = f16(w["wl2"])
    m["wm1"] = f16(np.asarray(w["wm1"]).reshape(8, 128, 512).transpose(1, 0, 2))
    m["wm2"] = f16(np.asarray(w["wm2"]).reshape(4, 128, 256).transpose(1, 0, 2))
    m["wout"] = f16(np.asarray(w["wout"]).reshape(2, 128, 2).transpose(1, 0, 2))
    m["rep4"] = _rep_mat(4)
    m["rep8"] = _rep_mat(8)
    m["b1a"] = f32(w["b1a"].reshape(64, 1))
    m["b1b"] = f32(w["b1b"].reshape(64, 1))
    m["b1c"] = f32(w["b1c"].reshape(64, 1))
    m["s1c"] = f32(w["s1c"].reshape(64, 1))
    m["s1c2"] = f32(w["s1c2"].reshape(64, 1))
    m["b2"] = f32(w["b2"].reshape(128, 1))
    m["bl"] = f32(np.asarray(w["bl"]).reshape(8, 128).T)
    m["bm1"] = f32(np.asarray(w["bm1"]).reshape(4, 128).T)
    m["bm2"] = f32(np.asarray(w["bm2"]).reshape(2, 128).T)
    m["bout"] = f32(w["bout"].reshape(2, 1))
    return m


def _build_in_maps(inputs):
    w = _fold_weights(inputs)
    wm = _weight_maps(w)
    pos64 = np.asarray(inputs["pos"], dtype=np.float64)  # [B, N, 3]
    B = pos64.shape[0]
    posT = np.zeros((B, 20, pos64.shape[1]), dtype=np.float32)
    xt = pos64.transpose(0, 2, 1)                        # [B, 3, N]
    posT[:, 0:3] = xt
    posT[:, 3] = -np.sum(pos64 * pos64, axis=2)          # -|x|^2
    posT[:, 16:19] = xt
    posT[:, 19] = 0.5
    in_maps = []
    for c in range(NCORES):
        m = dict(wm)
        m["posT"] = np.ascontiguousarray(posT[NG * c:NG * (c + 1)])
        in_maps.append(m)
    return in_maps


def kernel(**inputs):
    if "nc" not in _CACHE:
        _CACHE["nc"] = build_nc()
    nc = _CACHE["nc"]

    in_maps = _build_in_maps(inputs)
    B = np.asarray(inputs["pos"]).shape[0]

    res = bass_utils.run_bass_kernel_spmd(nc, in_maps, core_ids=list(range(NCORES)))
    out = np.zeros((B, 2), dtype=np.float32)
    for c in range(NCORES):
        out[NG * c:NG * (c + 1)] = res.results[c]["out"].T
    return out
